# revision 9
# baseline (speedup 1.0000x reference)
"""Causal self-attention (GPT-style, B=2 S=2048 E=1024 H=16) on 8 trn2 cores.

Sharding: data-parallel over batch (2) x tensor-parallel over heads (4 heads
per core).  Core c handles batch c//4 and heads 4*(c%4) .. +4.  Each core
computes a partial output projection (its 256 head-dims against the matching
W_proj rows) in bf16; the host sums the 4 partials per batch and adds b_proj.

v2: fp8e4m3 everywhere on the PE with DoubleRow perf mode (0.5 cycles/row,
2 contraction planes per instruction) for QKV / V / PV / projection; scores
stay plain fp8 (contraction is only 64 = one head's d).  Weights are scaled
x64 on the host so fp8 quantization stays out of the subnormal range; the
combined /4096 falls out in the exp scale and the final projection copy.
exp output is boosted x16 (bias=ln16) for better fp8 resolution of small
probabilities; the softmax denominator (ones-column of V) cancels it.

Overlap: DMAs are batched (14 per rep instead of 68 - each dma_start costs
~625ns of serialized HWDGE issue); the first QKV matmul only needs the first
k-pair of W_qk and x, which are the first two DMAs issued.  Attention's
score->exp->PV units interleave with the next chunk's QKV matmuls as in v1.
Normalization copies the PV psum to SBUF immediately (releasing the psum for
the next head-pair) and runs reciprocal+broadcast+multiply off the critical
path.  Engine split: PE matmuls, ACT exp (bottleneck), DVE casts/masks/
normalize, Pool broadcast + projection-psum copies + strip memsets.
"""

import os

import numpy as np
import ml_dtypes

import concourse.bass as bass
import concourse.tile as tile
from concourse import bacc, mybir
from concourse import bass_utils

F32 = mybir.dt.float32
FP8 = mybir.dt.float8e4  # unused (accuracy gate)
BF16 = mybir.dt.bfloat16
DR = mybir.MatmulPerfMode.DoubleRow

B, S, E, H = 2, 2048, 1024, 16
D = 64            # head dim
NCORES = 8
HPC = 4           # heads per core
DQ = HPC * D      # 256: per-core q/k/v width
KT = 8            # number of 128-row K tiles over E
P = 128
NEG = -1.0e30
WS = 1.0                                   # no prescale needed for bf16
SCALE = float(D) ** -0.5 / (WS * WS)       # exp input scale
EBIAS = 0.0                                # no exp boost needed for bf16
PSCALE = 1.0 / (WS * WS)                   # projection psum -> y scale

USE_FP32R = False  # legacy flag read by test.py banner

_PROGRAM_CACHE = {}


def build_program(reps=1):
    """Build + compile the per-core Tile program (cached per process)."""
    if reps in _PROGRAM_CACHE:
        return _PROGRAM_CACHE[reps]

    nc = bacc.Bacc("TRN2", target_bir_lowering=False, debug=False)

    xt = nc.dram_tensor("xt", [P, 4 * KT * 512], BF16, kind="ExternalInput")
    wqk = nc.dram_tensor("wqk", [P, KT * 512], BF16, kind="ExternalInput")
    wv = nc.dram_tensor("wv", [P, KT * 260], BF16, kind="ExternalInput")
    wp = nc.dram_tensor("wp", [P, 2 * E], BF16, kind="ExternalInput")
    cst = nc.dram_tensor("cst", [P, 524], F32, kind="ExternalInput")
    y = nc.dram_tensor("y", [S, E], BF16, kind="ExternalOutput")

    with tile.TileContext(nc) as tc:
        for rep in range(reps):
            _emit_rep(nc, tc, rep, xt, wqk, wv, wp, cst, y)

    nc.compile()
    _PROGRAM_CACHE[reps] = nc
    return nc


def _emit_rep(nc, tc, rep, xt, wqk, wv, wp, cst, y):
    Exp = mybir.ActivationFunctionType.Exp
    R = f"r{rep}_"
    with (
        tc.tile_pool(name=R + "consts", bufs=1) as consts,
        tc.tile_pool(name=R + "xin", bufs=1) as xin,
        tc.tile_pool(name=R + "work", bufs=1) as work,
    ):
        wqk_sb = consts.tile([P, KT * 512], BF16)
        wv_sb = consts.tile([P, KT * 260], BF16)
        wp_sb = consts.tile([P, 2 * E], BF16)
        cst_sb = consts.tile([P, 524], F32)
        # qkt: [d, s] m-blocks (Q01 | Q23 | K01 | K23), 2048 cols each.
        qkt_sb = consts.tile([P, 4 * S], BF16)
        # v: 16 s-blocks of [128, 4 heads * 65] (65th col = ones).
        v_sb = consts.tile([P, 16 * 260], BF16)
        # outT: [d, s] per head-pair (rows 0:64 head even, 64:128 odd).
        out_sb = consts.tile([P, 2 * S], BF16)

        bqk = cst_sb[:, 0:4]
        bv = cst_sb[:, 4:264]
        msk3 = cst_sb[:, 264:520].rearrange("p (h c) -> p h c", h=2)
        ebias = cst_sb[:, 520:521]

        wqk3 = wqk_sb[:].rearrange("p (k c) -> p k c", k=KT)
        wv3 = wv_sb[:].rearrange("p (k c) -> p k c", k=KT)
        wp3 = wp_sb[:].rearrange("p (t e) -> p t e", t=2)
        v4 = v_sb[:].rearrange("p (s h c) -> p s h c", s=16, h=HPC)

        # first k-pair of x for chunk 0 is its own tile so the first QKV
        # matmul's DMA dependency is just the first two (small) transfers
        xa = xin.tile([P, 1024], BF16, name="xa", tag="xa", bufs=1)
        xb = xin.tile([P, 3072], BF16, name="xb", tag="xb", bufs=1)
        xn = {}

        def xk(nch, k):
            """moving operand [128, 512] = x k-tile k of chunk nch"""
            if nch == 0:
                if k < 2:
                    return xa[:, k * 512 : (k + 1) * 512]
                return xb[:, (k - 2) * 512 : (k - 1) * 512]
            return xn[nch][:, k * 512 : (k + 1) * 512]

        with tc.tile_pool(name=R + "psum", space="PSUM", bufs=1) as ps4:
            proj_pending = []

            def _project(qc):
                ysb = work.tile([P, 4096], BF16, name="ysb", tag="ysb", bufs=2)
                o3 = out_sb[:].rearrange("p (t s) -> p t s", t=2)
                ydst = y[qc * 512 : (qc + 1) * 512, :].rearrange(
                    "(s p) e -> p s e", s=4
                )
                for sb4 in range(4):
                    sb = qc * 4 + sb4
                    for ec in range(2):
                        py = ps4.tile([P, 512], F32, name="py", tag="qv",
                                      bufs=2)
                        for t in range(2):
                            nc.tensor.matmul(
                                py[:],
                                o3[:, t, sb * P : (sb + 1) * P],
                                wp3[:, t, ec * 512 : (ec + 1) * 512],
                                start=(t == 0),
                                stop=(t == 1),
                            )
                        # GPSIMD cannot read PSUM on hw; copies run on DVE,
                        # with the last chunk's second half on ACT so the
                        # tail drains two copies at a time
                        ysl = ysb[:, sb4 * 1024 + ec * 512 :
                                  sb4 * 1024 + ec * 512 + 512]
                        if qc == 3 and ec == 1:
                            nc.scalar.activation(
                                ysl, py[:],
                                mybir.ActivationFunctionType.Copy,
                                scale=PSCALE,
                            )
                        else:
                            nc.vector.tensor_scalar_mul(ysl, py[:], PSCALE)
                    if qc == 3 and sb4 == 1:
                        nc.sync.dma_start(out=ydst[:, 0:2, :],
                                          in_=ysb[:, 0:2048])
                if qc == 3:
                    nc.sync.dma_start(out=ydst[:, 2:4, :], in_=ysb[:, 2048:4096])
                else:
                    nc.sync.dma_start(out=ydst, in_=ysb[:])

            def _pv(oA, oB, hp, pr, eP, rs, npr):
                """PV pair: V (s-blocks 2pr, 2pr+1) stationary, exp moving."""
                e4 = eP[:].rearrange("p (k h c) -> p k h c", k=2, h=2)
                for k2 in (0, 1):
                    kb = 2 * pr + k2
                    r = rs[k2]
                    for hi, o in ((0, oA), (1, oB)):
                        h = 2 * hp + hi
                        nc.tensor.matmul(
                            o[:, r:512],
                            v4[:, kb, h, :],
                            e4[:, k2, hi, r:512],
                            start=(kb == 0),
                            stop=(kb == 2 * npr - 1),
                            skip_group_check=True,
                        )

            def _attend_units(qc):
                npr = 2 * qc + 2
                for hp in range(2):
                    if hp == 1 and proj_pending:
                        _project(proj_pending.pop(0))
                    qcol = hp * S
                    kcol = (2 + hp) * S
                    oA = ps4.tile([65, 512], F32, name="oA", tag="oA", bufs=1)
                    oB = ps4.tile([65, 512], F32, name="oB", tag="oB", bufs=1)
                    pending = []
                    for pr in range(npr):
                        eP = work.tile([P, 2048], BF16, name="eP", tag="eP",
                                       bufs=4)
                        rs = []
                        for k2 in (0, 1):
                            kb = 2 * pr + k2
                            j = kb - 4 * qc
                            r = max(0, j) * P
                            rs.append(r)
                            sAB = ps4.tile([P, 1024], F32, name="sAB",
                                           tag="sAB", bufs=2)
                            s3 = sAB[:].rearrange("p (h c) -> p h c", h=2)
                            for h in (0, 1):
                                nc.tensor.matmul(
                                    sAB[:, h * 512 + r : (h + 1) * 512],
                                    qkt_sb[h * 64 : (h + 1) * 64,
                                           kcol + kb * P : kcol + (kb + 1) * P],
                                    qkt_sb[h * 64 : (h + 1) * 64,
                                           qcol + qc * 512 + r : qcol + qc * 512 + 512],
                                    start=True,
                                    stop=True,
                                )
                            if j >= 0:
                                nc.vector.tensor_add(
                                    s3[:, :, r : r + P], s3[:, :, r : r + P],
                                    msk3[:, :, :],
                                )
                            eh = eP[:, k2 * 1024 : (k2 + 1) * 1024].rearrange(
                                "p (h c) -> p h c", h=2
                            )
                            nc.scalar.activation(
                                eh[:, :, r:512], s3[:, :, r:512], Exp,
                                scale=SCALE, bias=ebias,
                            )
                        pending.append((pr, eP, rs))
                        if len(pending) > 1:
                            _pv(oA, oB, hp, *pending.pop(0), npr)
                        yield
                    for pend in pending:
                        _pv(oA, oB, hp, *pend, npr)

                    # normalize: copy psum out fast (frees oA/oB for the next
                    # head-pair), then reciprocal/broadcast/mul from the copy.
                    last = qc == 3 and hp == 1
                    if last:
                        srcA, srcB = oA, oB
                    else:
                        nAB = work.tile([65, 1024], F32, name="nAB", tag="nAB",
                                        bufs=2)
                        nc.vector.tensor_copy(nAB[:, 0:512], oA[:])
                        nc.vector.tensor_copy(nAB[:, 512:1024], oB[:])
                        srcA, srcB = nAB[:, 0:512], nAB[:, 512:1024]
                    rAB = work.tile([1, 1024], F32, name="rAB", tag="rAB",
                                    bufs=2)
                    sbAB = work.tile([64, 1024], F32, name="sbAB", tag="sbAB",
                                     bufs=2)
                    nc.vector.reciprocal(rAB[:, 0:512], srcA[64:65, :])
                    nc.gpsimd.partition_broadcast(sbAB[:, 0:512], rAB[:, 0:512])
                    nc.vector.reciprocal(rAB[:, 512:1024], srcB[64:65, :])
                    nc.vector.tensor_mul(
                        out_sb[0:64, hp * S + qc * 512 : hp * S + qc * 512 + 512],
                        srcA[0:64, :],
                        sbAB[:, 0:512],
                    )
                    nc.gpsimd.partition_broadcast(sbAB[:, 512:1024],
                                                  rAB[:, 512:1024])
                    nc.vector.tensor_mul(
                        out_sb[64:128, hp * S + qc * 512 : hp * S + qc * 512 + 512],
                        srcB[0:64, :],
                        sbAB[:, 512:1024],
                    )
                    yield

                proj_pending.append(qc)

            attend_q = []

            def _advance(n):
                done = 0
                while attend_q and done < n:
                    try:
                        next(attend_q[0])
                        done += 1
                    except StopIteration:
                        attend_q.pop(0)

            ADV = (0, 1, 2, 2)
            for nch in range(4):
                if nch == 0:
                    nc.sync.dma_start(out=wqk_sb[:, 0:512], in_=wqk[:, 0:512])
                    nc.sync.dma_start(out=xa[:, 0:512], in_=xt[:, 0:512])
                    nc.sync.dma_start(out=wqk_sb[:, 512:1024],
                                      in_=wqk[:, 512:1024])
                    nc.sync.dma_start(out=xa[:, 512:1024], in_=xt[:, 512:1024])
                    for kp in range(1, 4):
                        nc.sync.dma_start(
                            out=wqk_sb[:, kp * 1024 : (kp + 1) * 1024],
                            in_=wqk[:, kp * 1024 : (kp + 1) * 1024],
                        )
                        nc.sync.dma_start(
                            out=xb[:, (kp - 1) * 1024 : kp * 1024],
                            in_=xt[:, kp * 1024 : (kp + 1) * 1024],
                        )
                    nc.sync.dma_start(out=wv_sb[:], in_=wv[:])
                    nc.sync.dma_start(out=cst_sb[:], in_=cst[:])
                else:
                    t = xin.tile([P, 4096], BF16, name=f"xn{nch}", tag="xn",
                                 bufs=2)
                    xn[nch] = t
                    nc.sync.dma_start(
                        out=t[:], in_=xt[:, nch * 4096 : (nch + 1) * 4096]
                    )
                    if nch == 1:
                        nc.sync.dma_start(out=wp_sb[:], in_=wp[:])
                # Q^T / K^T m-blocks: weights stationary -> output [d, s].
                # Two half-passes of 2 m-blocks so the first matmuls only
                # need the first x/wqk k-tile (k-streaming at startup).
                for mh in range(2):
                    pss = [ps4.tile([P, 512], F32, name="ps_qkt", tag="qv",
                                    bufs=2) for _ in range(2)]
                    for k in range(KT):
                        for mi in range(2):
                            m = 2 * mh + mi
                            nc.tensor.matmul(
                                pss[mi][:],
                                wqk3[:, k, m * P : (m + 1) * P],
                                xk(nch, k),
                                start=(k == 0),
                                stop=(k == KT - 1),
                            )
                    for mi in range(2):
                        m = 2 * mh + mi
                        nc.vector.tensor_scalar_add(
                            qkt_sb[:, m * S + nch * 512 : m * S + nch * 512 + 512],
                            pss[mi][:],
                            bqk[:, m : m + 1],
                        )
                        _advance(ADV[nch])
                # V (+ ones column): x slices stationary -> [s, d] layout.
                for j in range(4):
                    sb = nch * 4 + j
                    psv = ps4.tile([P, 260], F32, name="ps_v", tag="qv",
                                   bufs=2)
                    for k in range(KT):
                        nc.tensor.matmul(
                            psv[:],
                            xk(nch, k)[:, j * P : (j + 1) * P],
                            wv3[:, k, :],
                            start=(k == 0),
                            stop=(k == KT - 1),
                        )
                    nc.vector.tensor_add(
                        v_sb[:, sb * 260 : (sb + 1) * 260], psv[:], bv
                    )
                    _advance(ADV[nch])
                attend_q.append(_attend_units(nch))

            _advance(10 ** 9)
            for q_ in proj_pending:
                _project(q_)


def _to_sbuf_layout(a, cols):
    """[KT*128, cols] -> [128, KT*cols] with col block k = K-tile k."""
    return np.ascontiguousarray(
        a.reshape(KT, P, cols).transpose(1, 0, 2).reshape(P, KT * cols)
    )


def _bf(a):
    return np.asarray(a, np.float32).astype(ml_dtypes.bfloat16)


def _pack_all(x, W_attn, b_attn, W_proj):
    f32 = np.float32
    x = np.asarray(x, f32)
    W_attn = np.asarray(W_attn, f32)
    b_attn = np.asarray(b_attn, f32)
    W_proj = np.asarray(W_proj, f32)
    maps = []
    for core in range(NCORES):
        b, hs = core // 4, (core % 4) * HPC
        m = {}
        # x: [p, nch*4096 + k*512 + s] = x[b, nch*512+s, k*128+p]
        xT = np.ascontiguousarray(x[b].T)                  # [E, S]
        m["xt"] = _bf(
            xT.reshape(KT, P, 4, 512).transpose(1, 2, 0, 3).reshape(P, -1)
        )
        wq = W_attn[:, hs * D : hs * D + DQ]
        wk = W_attn[:, E + hs * D : E + hs * D + DQ]
        m["wqk"] = _bf(
            _to_sbuf_layout(np.concatenate([wq, wk], axis=1) * WS, 512)
        )
        wv_heads = W_attn[:, 2 * E + hs * D : 2 * E + hs * D + DQ].reshape(
            E, HPC, D
        )
        wva = np.zeros((E, HPC, 65), f32)
        wva[:, :, :D] = wv_heads * WS
        m["wv"] = _bf(_to_sbuf_layout(wva.reshape(E, HPC * 65), 260))
        m["wp"] = _bf(
            (W_proj[hs * D : hs * D + DQ, :] * WS)
            .reshape(2, P, E)
            .transpose(1, 0, 2)
            .reshape(P, 2 * E)
        )
        cstm = np.zeros((P, 524), f32)
        cstm[:, 520] = EBIAS
        cstm[:, 0:4] = np.stack(
            [
                b_attn[hs * D : hs * D + P],
                b_attn[hs * D + P : hs * D + DQ],
                b_attn[E + hs * D : E + hs * D + P],
                b_attn[E + hs * D + P : E + hs * D + DQ],
            ],
            axis=1,
        ) * WS
        bv_row = np.zeros((HPC, 65), f32)
        bv_row[:, :D] = (
            b_attn[2 * E + hs * D : 2 * E + hs * D + DQ].reshape(HPC, D) * WS
        )
        bv_row[:, D] = 1.0
        cstm[:, 4:264] = np.broadcast_to(bv_row.reshape(1, 260), (P, 260))
        pgrid = np.arange(P)[:, None]
        fgrid = np.arange(P)[None, :]
        band = np.where(pgrid <= fgrid, 0.0, NEG).astype(f32)
        cstm[:, 264:520] = np.concatenate([band, band], axis=1)
        m["cst"] = cstm
        maps.append(m)
    return maps


LAST_RESULTS = None


def kernel(x, W_attn, b_attn, W_proj, b_proj):
    global LAST_RESULTS
    b_proj = np.asarray(b_proj, dtype=np.float32)

    nc = build_program()
    in_maps = _pack_all(x, W_attn, b_attn, W_proj)
    res = bass_utils.run_bass_kernel_spmd(nc, in_maps, list(range(NCORES)))
    LAST_RESULTS = res

    y = np.zeros((B, S, E), np.float32)
    for b in range(B):
        acc = np.zeros((S, E), np.float32)
        for i in range(4):
            acc += res.results[4 * b + i]["y"].astype(np.float32)
        y[b] = acc + b_proj[None, :]
    return y
